# revision 4
# baseline (speedup 1.0000x reference)
"""Mixture memory model (retrieval_knn) on 8 Trainium2 NeuronCores.

Device kernel (raw Bass, SPMD — same program on all 8 cores): each core
streams its M/8 = 2048-row shard of the memory bank in 16 tiles of
[128, 4096] and produces, per row,
  sqn[i] = sum_d mem[i, d]^2        (ScalarE: activation Square + accum)
  dot[i] = sum_d mem[i, d]*rep[d]   (VectorE: tensor_tensor_reduce mult/add)
Each compute engine reads every element exactly once, overlapped with the
DMA stream, so the kernel sits on the HBM roofline (32 MiB/core).

Host combine: sq_dist = sqn - 2*dot + ||rep||^2, then the per-row Gaussian
log-pdf, logsumexp, max, and decision over all 16384 rows in float64.

Raw Bass (not Tile) because this image's walrus encodes at most one sync
wait per instruction; Tile's kernel-tail drain emits multi-wait Drains.
"""

import contextlib

import numpy as np

_NOISE_SLOPE = 1.0
_NOISE_OFFSET = 0.001
_CRITERION = 0.5

_D = 4096
_M = 16384
_N_CORES = 8
_P = 128
_M_SHARD = _M // _N_CORES  # 2048
_N_TILES = _M_SHARD // _P  # 16
_NBUF = 6

_NC_CACHE = None


def _build_bass():
    import concourse.bass as bass
    from concourse import mybir

    nc = bass.Bass()
    f32 = mybir.dt.float32
    mem = nc.declare_dram_parameter("mem", [_M_SHARD, _D], f32, isOutput=False)
    rep = nc.declare_dram_parameter("rep", [_D], f32, isOutput=False)
    out = nc.declare_dram_parameter("out", [_P, 2 * _N_TILES], f32, isOutput=True)

    mem_v = mem[:].rearrange("(n p) d -> n p d", p=_P)
    rep_ap = rep[:]
    # rep replicated to all 128 partitions (partition step 0 on the DRAM side)
    rep_bcast = bass.AP(
        tensor=rep_ap.tensor,
        offset=rep_ap.offset,
        ap=[[0, _P]] + list(rep_ap.ap),
    )

    with contextlib.ExitStack() as ctx:
        mem_tiles = [
            ctx.enter_context(nc.sbuf_tensor(f"mem_tile{b}", [_P, _D], f32))
            for b in range(_NBUF)
        ]
        rep_b = ctx.enter_context(nc.sbuf_tensor("rep_b", [_P, _D], f32))
        act_scr = ctx.enter_context(nc.sbuf_tensor("act_scr", [_P, _D], f32))
        dve_scr = ctx.enter_context(nc.sbuf_tensor("dve_scr", [_P, 1], f32))
        sq_res = ctx.enter_context(nc.sbuf_tensor("sq_res", [_P, _N_TILES], f32))
        dot_res = ctx.enter_context(nc.sbuf_tensor("dot_res", [_P, _N_TILES], f32))

        rep_sem = ctx.enter_context(nc.semaphore(name="rep_sem"))
        dma_sem = ctx.enter_context(nc.semaphore(name="dma_sem"))
        act_sem = ctx.enter_context(nc.semaphore(name="act_sem"))
        dve_sem = ctx.enter_context(nc.semaphore(name="dve_sem"))
        block = ctx.enter_context(nc.Block())

        @block.sync
        def _(sync):
            sync.dma_start(out=rep_b[:], in_=rep_bcast).then_inc(rep_sem, 16)
            for j in range(_N_TILES):
                if j >= _NBUF:
                    sync.wait_ge(act_sem, j - _NBUF + 1)
                    sync.wait_ge(dve_sem, j - _NBUF + 1)
                sync.dma_start(out=mem_tiles[j % _NBUF][:], in_=mem_v[j]).then_inc(
                    dma_sem, 16
                )
            sync.wait_ge(act_sem, _N_TILES)
            sync.wait_ge(dve_sem, _N_TILES)
            sync.dma_start(out=out[:, :_N_TILES], in_=sq_res[:]).then_inc(dma_sem, 16)
            sync.dma_start(out=out[:, _N_TILES:], in_=dot_res[:]).then_inc(dma_sem, 16)
            sync.wait_ge(dma_sem, 16 * (_N_TILES + 2))

        @block.scalar
        def _(scalar):
            for j in range(_N_TILES):
                scalar.wait_ge(dma_sem, 16 * (j + 1))
                nc.scalar.activation(
                    act_scr[:],
                    mem_tiles[j % _NBUF][:],
                    mybir.ActivationFunctionType.Square,
                    accum_out=sq_res[:, j : j + 1],
                ).then_inc(act_sem, 1)

        @block.vector
        def _(vector):
            vector.wait_ge(rep_sem, 16)
            for j in range(_N_TILES):
                vector.wait_ge(dma_sem, 16 * (j + 1))
                nc.vector.scalar_tensor_tensor(
                    out=dve_scr.broadcast_to((_P, _D)),
                    in0=mem_tiles[j % _NBUF][:],
                    scalar=1.0,
                    in1=rep_b[:],
                    op0=mybir.AluOpType.mult,
                    op1=mybir.AluOpType.mult,
                    accum_out=dot_res[:, j : j + 1],
                ).then_inc(dve_sem, 1)

    return nc


def _get_nc():
    global _NC_CACHE
    if _NC_CACHE is None:
        _NC_CACHE = _build_bass()
    return _NC_CACHE


def _run(rep, memory_bank, trace=False):
    from concourse.bass_utils import run_bass_kernel_spmd

    rep = np.ascontiguousarray(np.asarray(rep, dtype=np.float32))
    mem = np.ascontiguousarray(np.asarray(memory_bank, dtype=np.float32))
    assert rep.shape == (_D,) and mem.shape == (_M, _D)

    nc = _get_nc()
    in_maps = [
        {"mem": mem[i * _M_SHARD : (i + 1) * _M_SHARD], "rep": rep}
        for i in range(_N_CORES)
    ]
    res = run_bass_kernel_spmd(nc, in_maps, list(range(_N_CORES)), trace=trace)

    sqn_parts = []
    dot_parts = []
    for i in range(_N_CORES):
        o = res.results[i]["out"]  # [128, 32]; column j = row-tile j of the shard
        sqn_parts.append(o[:, :_N_TILES].T.reshape(-1))  # row j*128+p -> o[p, j]
        dot_parts.append(o[:, _N_TILES:].T.reshape(-1))
    sqn = np.concatenate(sqn_parts).astype(np.float64)
    dot = np.concatenate(dot_parts).astype(np.float64)

    rep64 = rep.astype(np.float64)
    sq_dist = sqn - 2.0 * dot + float(rep64 @ rep64)

    t = np.arange(_M, 0, -1, dtype=np.float64)
    var = _NOISE_SLOPE * t + _NOISE_OFFSET
    log_probs = -0.5 * (_D * np.log(2.0 * np.pi * var) + sq_dist / var)
    mx = log_probs.max()
    lse = mx + np.log(np.exp(log_probs - mx).sum())
    log_likelihood = lse - np.log(float(_M))
    threshold = np.log(_CRITERION) + mx
    decision = np.float32(1.0) if log_likelihood >= threshold else np.float32(0.0)

    out = (
        np.array([decision], dtype=np.float32),
        np.asarray(log_likelihood, dtype=np.float32),
        np.asarray(threshold, dtype=np.float32),
    )
    return out, res


def kernel(rep, memory_bank):
    out, _ = _run(rep, memory_bank, trace=False)
    return out


# revision 9
# speedup vs baseline: 1.1278x; 1.1278x over previous
"""Mixture memory model (retrieval_knn) on 8 Trainium2 NeuronCores.

Device kernel (raw Bass, SPMD — same program on all 8 cores): each core
streams its M/8 = 2048-row shard of the memory bank in 16 tiles of
[128, 4096] and produces, per row,
  sqn[i] = sum_d mem[i, d]^2        (ScalarE: activation Square + accum)
  dot[i] = sum_d mem[i, d]*rep[d]   (VectorE: tensor_tensor_reduce mult/add)
Each compute engine reads every element exactly once, overlapped with the
DMA stream, so the kernel sits on the HBM roofline (32 MiB/core).

Host combine: sq_dist = sqn - 2*dot + ||rep||^2, then the per-row Gaussian
log-pdf, logsumexp, max, and decision over all 16384 rows in float64.

Raw Bass (not Tile) because this image's walrus encodes at most one sync
wait per instruction; Tile's kernel-tail drain emits multi-wait Drains.
"""

import contextlib

import numpy as np

_NOISE_SLOPE = 1.0
_NOISE_OFFSET = 0.001
_CRITERION = 0.5

_D = 4096
_M = 16384
_N_CORES = 8
_P = 128
_M_SHARD = _M // _N_CORES  # 2048
_N_TILES = _M_SHARD // _P  # 16
_NBUF = 6

_NC_CACHE = None


def _build_bass():
    import concourse.bass as bass
    from concourse import mybir

    nc = bass.Bass()
    f32 = mybir.dt.float32
    mem = nc.declare_dram_parameter("mem", [_M_SHARD, _D], f32, isOutput=False)
    rep = nc.declare_dram_parameter("rep", [_D], f32, isOutput=False)
    out = nc.declare_dram_parameter("out", [_P, 2 * _N_TILES], f32, isOutput=True)

    mem_v = mem[:].rearrange("(n p) d -> n p d", p=_P)
    rep_ap = rep[:]

    with contextlib.ExitStack() as ctx:
        mem_tiles = [
            ctx.enter_context(nc.sbuf_tensor(f"mem_tile{b}", [_P, _D], f32))
            for b in range(_NBUF)
        ]
        rep_b = ctx.enter_context(nc.sbuf_tensor("rep_b", [_P, _D], f32))
        act_scr = ctx.enter_context(nc.sbuf_tensor("act_scr", [_P, _D], f32))
        act_warm = ctx.enter_context(nc.sbuf_tensor("act_warm", [_P, 1], f32))
        dve_scr = ctx.enter_context(nc.sbuf_tensor("dve_scr", [_P, 1], f32))
        sq_res = ctx.enter_context(nc.sbuf_tensor("sq_res", [_P, _N_TILES], f32))
        dot_res = ctx.enter_context(nc.sbuf_tensor("dot_res", [_P, _N_TILES], f32))

        rep_row_sem = ctx.enter_context(nc.semaphore(name="rep_row_sem"))
        rep_sem = ctx.enter_context(nc.semaphore(name="rep_sem"))
        dma_sem = ctx.enter_context(nc.semaphore(name="dma_sem"))
        act_sem = ctx.enter_context(nc.semaphore(name="act_sem"))
        dve_sem = ctx.enter_context(nc.semaphore(name="dve_sem"))
        block = ctx.enter_context(nc.Block())

        # rep replicated to all 128 partitions (partition step 0 on the DRAM
        # read side). Issued from gpsimd's SWDGE queue so the sync HWDGE
        # queue carries only the 2 MiB tile loads.
        rep_bcast = bass.AP(
            tensor=rep_ap.tensor,
            offset=rep_ap.offset,
            ap=[[0, _P]] + list(rep_ap.ap),
        )

        @block.sync
        def _(sync):
            for j in range(_N_TILES):
                if j >= _NBUF:
                    sync.wait_ge(act_sem, j - _NBUF + 1)
                    sync.wait_ge(dve_sem, j - _NBUF + 1)
                sync.dma_start(out=mem_tiles[j % _NBUF][:], in_=mem_v[j]).then_inc(
                    dma_sem, 16
                )
            sync.wait_ge(act_sem, _N_TILES)
            sync.wait_ge(dve_sem, _N_TILES)
            sync.dma_start(out=out[:, :_N_TILES], in_=sq_res[:]).then_inc(dma_sem, 16)
            sync.dma_start(out=out[:, _N_TILES:], in_=dot_res[:]).then_inc(dma_sem, 16)
            sync.wait_ge(dma_sem, 16 * (_N_TILES + 2))

        @block.gpsimd
        def _(gpsimd):
            gpsimd.dma_start(out=rep_b[:], in_=rep_bcast).then_inc(rep_sem, 16)

        @block.scalar
        def _(scalar):
            # Warmup: pull the Square PWP table into ACT before data arrives.
            nc.scalar.activation(
                act_warm[:],
                act_warm[:],
                mybir.ActivationFunctionType.Square,
            )
            for j in range(_N_TILES):
                scalar.wait_ge(dma_sem, 16 * (j + 1))
                nc.scalar.activation(
                    act_scr[:],
                    mem_tiles[j % _NBUF][:],
                    mybir.ActivationFunctionType.Square,
                    accum_out=sq_res[:, j : j + 1],
                ).then_inc(act_sem, 1)

        @block.vector
        def _(vector):
            vector.wait_ge(rep_sem, 16)
            for j in range(_N_TILES):
                vector.wait_ge(dma_sem, 16 * (j + 1))
                nc.vector.scalar_tensor_tensor(
                    out=dve_scr.broadcast_to((_P, _D)),
                    in0=mem_tiles[j % _NBUF][:],
                    scalar=1.0,
                    in1=rep_b[:],
                    op0=mybir.AluOpType.mult,
                    op1=mybir.AluOpType.mult,
                    accum_out=dot_res[:, j : j + 1],
                ).then_inc(dve_sem, 1)

    return nc


def _get_nc():
    global _NC_CACHE
    if _NC_CACHE is None:
        _NC_CACHE = _build_bass()
    return _NC_CACHE


def _run(rep, memory_bank, trace=False):
    from concourse.bass_utils import run_bass_kernel_spmd

    rep = np.ascontiguousarray(np.asarray(rep, dtype=np.float32))
    mem = np.ascontiguousarray(np.asarray(memory_bank, dtype=np.float32))
    assert rep.shape == (_D,) and mem.shape == (_M, _D)

    nc = _get_nc()
    in_maps = [
        {"mem": mem[i * _M_SHARD : (i + 1) * _M_SHARD], "rep": rep}
        for i in range(_N_CORES)
    ]
    res = run_bass_kernel_spmd(nc, in_maps, list(range(_N_CORES)), trace=trace)

    sqn_parts = []
    dot_parts = []
    for i in range(_N_CORES):
        o = res.results[i]["out"]  # [128, 32]; column j = row-tile j of the shard
        sqn_parts.append(o[:, :_N_TILES].T.reshape(-1))  # row j*128+p -> o[p, j]
        dot_parts.append(o[:, _N_TILES:].T.reshape(-1))
    sqn = np.concatenate(sqn_parts).astype(np.float64)
    dot = np.concatenate(dot_parts).astype(np.float64)

    rep64 = rep.astype(np.float64)
    sq_dist = sqn - 2.0 * dot + float(rep64 @ rep64)

    t = np.arange(_M, 0, -1, dtype=np.float64)
    var = _NOISE_SLOPE * t + _NOISE_OFFSET
    log_probs = -0.5 * (_D * np.log(2.0 * np.pi * var) + sq_dist / var)
    mx = log_probs.max()
    lse = mx + np.log(np.exp(log_probs - mx).sum())
    log_likelihood = lse - np.log(float(_M))
    threshold = np.log(_CRITERION) + mx
    decision = np.float32(1.0) if log_likelihood >= threshold else np.float32(0.0)

    out = (
        np.array([decision], dtype=np.float32),
        np.asarray(log_likelihood, dtype=np.float32),
        np.asarray(threshold, dtype=np.float32),
    )
    return out, res


def kernel(rep, memory_bank):
    out, _ = _run(rep, memory_bank, trace=False)
    return out


# revision 10
# speedup vs baseline: 1.2120x; 1.0746x over previous
"""Mixture memory model (retrieval_knn) on 8 Trainium2 NeuronCores.

Device kernel (raw Bass, SPMD — same program on all 8 cores): each core
streams its M/8 = 2048-row shard of the memory bank in 16 tiles of
[128, 4096] and produces, per row,
  sqn[i] = sum_d mem[i, d]^2        (ScalarE: activation Square + accum)
  dot[i] = sum_d mem[i, d]*rep[d]   (VectorE: scalar_tensor_tensor + accum)
Each compute engine reads every element exactly once, overlapped with the
DMA stream, so the kernel sits on the HBM/DMA-port roofline (32 MiB/core).

rep is loaded once as a single [1, 4096] row (16 KiB) and replicated to
all 128 partitions by the idle TensorEngine as an outer product
ones[128] x rep into PSUM — zero DMA-port traffic — and the VectorE
reads it directly from PSUM. The last tile is split in half so the tail
after the final DMA byte is only half a tile of compute.

Host combine: sq_dist = sqn - 2*dot + ||rep||^2, then the per-row Gaussian
log-pdf, logsumexp, max, and decision over all 16384 rows in float64.

Raw Bass (not Tile) because this image's walrus encodes at most one sync
wait per instruction; Tile's kernel-tail drain emits multi-wait Drains.
"""

import contextlib

import numpy as np

_NOISE_SLOPE = 1.0
_NOISE_OFFSET = 0.001
_CRITERION = 0.5

_D = 4096
_M = 16384
_N_CORES = 8
_P = 128
_M_SHARD = _M // _N_CORES  # 2048
_N_TILES = _M_SHARD // _P  # 16
_NBUF = 6
_NFULL = _N_TILES - 1  # tiles processed as one [128, 4096] op
_NCOL = _NFULL + 2  # result columns per engine (15 full + 2 halves)
_HALF = _D // 2

_NC_CACHE = None


def _build_bass():
    import concourse.bass as bass
    from concourse import mybir

    nc = bass.Bass(enable_partition_id=False)
    f32 = mybir.dt.float32
    mem = nc.declare_dram_parameter("mem", [_M_SHARD, _D], f32, isOutput=False)
    rep = nc.declare_dram_parameter("rep", [_D], f32, isOutput=False)
    out = nc.declare_dram_parameter("out", [_P, 2 * _NCOL], f32, isOutput=True)

    mem_v = mem[:].rearrange("(n p) d -> n p d", p=_P)

    with contextlib.ExitStack() as ctx:
        mem_tiles = [
            ctx.enter_context(nc.sbuf_tensor(f"mem_tile{b}", [_P, _D], f32))
            for b in range(_NBUF)
        ]
        rep_row = ctx.enter_context(nc.sbuf_tensor("rep_row", [1, _D], f32))
        ones_t = ctx.enter_context(nc.sbuf_tensor("ones_t", [1, _P], f32))
        act_scr = ctx.enter_context(nc.sbuf_tensor("act_scr", [_P, _D], f32))
        act_warm = ctx.enter_context(nc.sbuf_tensor("act_warm", [_P, 1], f32))
        dve_scr = ctx.enter_context(nc.sbuf_tensor("dve_scr", [_P, 1], f32))
        sq_res = ctx.enter_context(nc.sbuf_tensor("sq_res", [_P, _NCOL], f32))
        dot_res = ctx.enter_context(nc.sbuf_tensor("dot_res", [_P, _NCOL], f32))
        rep_ps = ctx.enter_context(nc.psum_tensor("rep_ps", [_P, _D], f32))

        r1 = ctx.enter_context(nc.semaphore(name="r1"))
        m1 = ctx.enter_context(nc.semaphore(name="m1"))
        mm = ctx.enter_context(nc.semaphore(name="mm"))
        dma_sem = ctx.enter_context(nc.semaphore(name="dma_sem"))
        act_sem = ctx.enter_context(nc.semaphore(name="act_sem"))
        dve_sem = ctx.enter_context(nc.semaphore(name="dve_sem"))
        block = ctx.enter_context(nc.Block())

        @block.sync
        def _(sync):
            sync.dma_start(
                out=rep_row[:], in_=rep[:].rearrange("(o d) -> o d", o=1)
            ).then_inc(r1, 16)
            for j in range(_NFULL):
                if j >= _NBUF:
                    sync.wait_ge(act_sem, j - _NBUF + 1)
                    sync.wait_ge(dve_sem, j - _NBUF + 1)
                sync.dma_start(out=mem_tiles[j % _NBUF][:], in_=mem_v[j]).then_inc(
                    dma_sem, 16
                )
            # last tile in two halves for a shorter tail
            jl = _NFULL
            sync.wait_ge(act_sem, jl - _NBUF + 1)
            sync.wait_ge(dve_sem, jl - _NBUF + 1)
            lt = mem_tiles[jl % _NBUF]
            sync.dma_start(out=lt[:, :_HALF], in_=mem_v[jl][:, :_HALF]).then_inc(
                dma_sem, 16
            )
            sync.dma_start(out=lt[:, _HALF:], in_=mem_v[jl][:, _HALF:]).then_inc(
                dma_sem, 16
            )
            # bulk store of the first 15 tiles' results
            sync.wait_ge(act_sem, _NFULL)
            sync.wait_ge(dve_sem, _NFULL)
            sync.dma_start(out=out[:, :_NFULL], in_=sq_res[:, :_NFULL]).then_inc(
                dma_sem, 16
            )
            sync.dma_start(
                out=out[:, _NCOL : _NCOL + _NFULL], in_=dot_res[:, :_NFULL]
            ).then_inc(dma_sem, 16)
            # final store of the last tile's two half-columns per engine
            sync.wait_ge(act_sem, _NFULL + 2)
            sync.wait_ge(dve_sem, _NFULL + 2)
            sync.dma_start(
                out=out[:, _NFULL:_NCOL], in_=sq_res[:, _NFULL:]
            ).then_inc(dma_sem, 16)
            sync.dma_start(
                out=out[:, _NCOL + _NFULL :], in_=dot_res[:, _NFULL:]
            ).then_inc(dma_sem, 16)
            sync.wait_ge(dma_sem, 16 * (_N_TILES + 1 + 4))

        @block.tensor
        def _(tensor):
            # Broadcast rep across partitions: ones[128] (x) rep outer
            # product into PSUM, 512 columns (one bank) per matmul.
            tensor.wait_ge(r1, 16)
            tensor.wait_ge(m1, 1)
            for k in range(_D // 512):
                nc.tensor.matmul(
                    rep_ps[:, k * 512 : (k + 1) * 512],
                    ones_t[:],
                    rep_row[:, k * 512 : (k + 1) * 512],
                    start=True,
                    stop=True,
                ).then_inc(mm, 1)

        @block.scalar
        def _(scalar):
            # Warmup: pull the Square PWP table into ACT before data arrives.
            nc.scalar.activation(
                act_warm[:],
                act_warm[:],
                mybir.ActivationFunctionType.Square,
            )
            for j in range(_NFULL):
                scalar.wait_ge(dma_sem, 16 * (j + 1))
                nc.scalar.activation(
                    act_scr[:],
                    mem_tiles[j % _NBUF][:],
                    mybir.ActivationFunctionType.Square,
                    accum_out=sq_res[:, j : j + 1],
                ).then_inc(act_sem, 1)
            lt = mem_tiles[_NFULL % _NBUF]
            for h in range(2):
                scalar.wait_ge(dma_sem, 16 * (_NFULL + 1 + h))
                nc.scalar.activation(
                    act_scr[:, :_HALF],
                    lt[:, h * _HALF : (h + 1) * _HALF],
                    mybir.ActivationFunctionType.Square,
                    accum_out=sq_res[:, _NFULL + h : _NFULL + h + 1],
                ).then_inc(act_sem, 1)

        @block.vector
        def _(vector):
            nc.vector.memset(ones_t[:], 1.0).then_inc(m1, 1)
            vector.wait_ge(mm, _D // 512)
            for j in range(_NFULL):
                vector.wait_ge(dma_sem, 16 * (j + 1))
                nc.vector.scalar_tensor_tensor(
                    out=dve_scr.broadcast_to((_P, _D)),
                    in0=mem_tiles[j % _NBUF][:],
                    scalar=1.0,
                    in1=rep_ps[:],
                    op0=mybir.AluOpType.mult,
                    op1=mybir.AluOpType.mult,
                    accum_out=dot_res[:, j : j + 1],
                ).then_inc(dve_sem, 1)
            lt = mem_tiles[_NFULL % _NBUF]
            for h in range(2):
                vector.wait_ge(dma_sem, 16 * (_NFULL + 1 + h))
                nc.vector.scalar_tensor_tensor(
                    out=dve_scr.broadcast_to((_P, _HALF)),
                    in0=lt[:, h * _HALF : (h + 1) * _HALF],
                    scalar=1.0,
                    in1=rep_ps[:, h * _HALF : (h + 1) * _HALF],
                    op0=mybir.AluOpType.mult,
                    op1=mybir.AluOpType.mult,
                    accum_out=dot_res[:, _NFULL + h : _NFULL + h + 1],
                ).then_inc(dve_sem, 1)

    return nc


def _get_nc():
    global _NC_CACHE
    if _NC_CACHE is None:
        _NC_CACHE = _build_bass()
    return _NC_CACHE


def _unpack(cols):
    """[128, _NCOL] per-engine result -> [2048] per-row values."""
    full = cols[:, :_NFULL].T.reshape(-1)  # rows 0 .. 15*128-1
    last = cols[:, _NFULL] + cols[:, _NFULL + 1]  # two half-sums
    return np.concatenate([full, last])


def _run(rep, memory_bank, trace=False):
    from concourse.bass_utils import run_bass_kernel_spmd

    rep = np.ascontiguousarray(np.asarray(rep, dtype=np.float32))
    mem = np.ascontiguousarray(np.asarray(memory_bank, dtype=np.float32))
    assert rep.shape == (_D,) and mem.shape == (_M, _D)

    nc = _get_nc()
    in_maps = [
        {"mem": mem[i * _M_SHARD : (i + 1) * _M_SHARD], "rep": rep}
        for i in range(_N_CORES)
    ]
    res = run_bass_kernel_spmd(nc, in_maps, list(range(_N_CORES)), trace=trace)

    sqn_parts = []
    dot_parts = []
    for i in range(_N_CORES):
        o = res.results[i]["out"].astype(np.float64)  # [128, 2*_NCOL]
        sqn_parts.append(_unpack(o[:, :_NCOL]))
        dot_parts.append(_unpack(o[:, _NCOL:]))
    sqn = np.concatenate(sqn_parts)
    dot = np.concatenate(dot_parts)

    rep64 = rep.astype(np.float64)
    sq_dist = sqn - 2.0 * dot + float(rep64 @ rep64)

    t = np.arange(_M, 0, -1, dtype=np.float64)
    var = _NOISE_SLOPE * t + _NOISE_OFFSET
    log_probs = -0.5 * (_D * np.log(2.0 * np.pi * var) + sq_dist / var)
    mx = log_probs.max()
    lse = mx + np.log(np.exp(log_probs - mx).sum())
    log_likelihood = lse - np.log(float(_M))
    threshold = np.log(_CRITERION) + mx
    decision = np.float32(1.0) if log_likelihood >= threshold else np.float32(0.0)

    out = (
        np.array([decision], dtype=np.float32),
        np.asarray(log_likelihood, dtype=np.float32),
        np.asarray(threshold, dtype=np.float32),
    )
    return out, res


def kernel(rep, memory_bank):
    out, _ = _run(rep, memory_bank, trace=False)
    return out
